# revision 1
# baseline (speedup 1.0000x reference)
"""ColorizationNet Trainium2 kernel (8 NeuronCores, SPMD, two phases).

Structure exploited: rows of the big FC input [4096, 32786] share an identical
x_conv prefix (32768 cols), so

    fc_in @ w1.T = x_conv @ w1[:, :32768].T  (one shared matvec, [304])
                 + [pos|chunks] @ w1[:, 32768:].T  ([4096,18] GEMM)

Sharding (core r of 8):
  - conv backbone row-sharded: core r produces the x_conv slice for pooled
    rows [4r, 4r+4) of every channel (halos via zero-padded input windows,
    out-of-image "phantom" rows masked to zero via activation scale).
  - shared matvec K-sharded to match (each core streams 1/8 of w1's big
    part, 5 MB, laid out so each SBUF partition's data is contiguous in
    DRAM).  Phase A outputs the 8 partials [304]; the host sums them
    (collectives are unavailable under the axon PJRT execution path).
  - phase B: patch FC sharded by patch row, core r handles patches
    [512r, 512(r+1)).

Convs use a banded-rows formulation: moving operand = input rows on SBUF
partitions (k = (row, c_in)), stationary = banded weight matrix with output
columns m = (s, rowpair, c_out) so the 2x2 maxpool's vertical pair is
partitions p / p+64 (one tensor_max) and the horizontal pair is a stride-2
free-dim pair.  Each layer's pooled activation is written by ScalarE
directly into the next layer's moving-window tiles (no DRAM round trips).
DMA count is minimized (one packed const tensor per phase) because each DMA
costs >1us of fixed sequencer/DGE overhead.
"""

import sys

for _p in ("/opt/trn_rl_repo",):
    if _p not in sys.path:
        sys.path.insert(0, _p)

import numpy as np
from contextlib import ExitStack

IMG = 256
CS = 4
G = 64
H1 = 304
H2 = 176
OUT = 48
NCORES = 8

# dtype knobs (mybir dtypes, set lazily in kernel()); float32 = exact,
# float32r = full-rate PE at reduced multiply precision
DT_MV = None
DT_FC = None
DT_CV = None

# phase-A packed const layout: [96 partitions, CA_W] fp32
#   s1  [18, 384]  at cols [0, 384)      (+ copy at rows 32..50 for block 2)
#   s2  [80, 384]  at cols [384, 768)
#   s3  [96, 384]  at cols [768, 1152)
#   mk1/bm1/mk2/bm2 [64, 3] at cols 1152/1155/1158/1161
#   bc3 [64, 1]    at col 1164
CA_W = 1165

# phase-B packed const layout: [128 partitions, CB_W] fp32
#   extrasT [18, 512] @0, w1eT [18, 304] @512, w2a/b [128, 176] @816/@992,
#   w2c [48, 176] @1168, w3a [128, 48] @1344, w3b [48, 48] @1392,
#   b2a [128, 1] @1440, b2b [48, 1] @1441, b3 [48, 1] @1442
CB_W = 1443


def _build_s1(c1_w):
    # [18, 3, 128]: rows i = in-row in window; cols m = s*64 + jp*8 + c
    s1 = np.zeros((18, 3, 128), np.float32)
    for dx in range(3):
        for s in range(2):
            for jp in range(8):
                j = 2 * jp + s
                for c in range(8):
                    m = s * 64 + jp * 8 + c
                    for dy in range(3):
                        s1[j + dy, dx, m] = c1_w[c, 0, dy, dx]
    return np.ascontiguousarray(s1.reshape(18, 3 * 128))


def _build_s2(c2_w):
    # [80, 3, 128]: rows k = delta*8 + ci (ci in 0..8); cols m = s*64+jp*16+co
    s2 = np.zeros((80, 3, 128), np.float32)
    for dx in range(3):
        for s in range(2):
            for jp in range(4):
                j2 = 2 * jp + s
                for co in range(16):
                    m = s * 64 + jp * 16 + co
                    for ci in range(8):
                        for dy in range(3):
                            s2[(j2 + dy) * 8 + ci, dx, m] = c2_w[co, ci, dy, dx]
    return np.ascontiguousarray(s2.reshape(80, 3 * 128))


def _build_s3(c3_w):
    # [96, 3, 128]: rows k = delta*16 + ci (ci in 0..16); cols m = s*64+jpp*32+co
    s3 = np.zeros((96, 3, 128), np.float32)
    for dx in range(3):
        for s in range(2):
            for jpp in range(2):
                j3 = 2 * jpp + s
                for co in range(32):
                    m = s * 64 + jpp * 32 + co
                    for ci in range(16):
                        for dy in range(3):
                            s3[(j3 + dy) * 16 + ci, dx, m] = c3_w[co, ci, dy, dx]
    return np.ascontiguousarray(s3.reshape(96, 3 * 128))


def _host_inputs(x, c1_w, c1_b, c2_w, c2_b, c3_w, c3_b, w1, b1, w2, b2, w3, b3):
    """Returns (in_maps_a, in_maps_b_partial, b1). Each phase-A map has
    'xs' [50,258], 'xs2' [34,258], 'ca' [96, CA_W], 'w1ps' [128, 9728].
    Each phase-B map has 'cb' [128, CB_W]; 'shc' [128, 3] is added after
    phase A."""
    x = np.asarray(x, np.float32).reshape(IMG, IMG)
    s1 = _build_s1(np.asarray(c1_w, np.float32))
    s2 = _build_s2(np.asarray(c2_w, np.float32))
    s3 = _build_s3(np.asarray(c3_w, np.float32))
    bc3 = np.tile(np.asarray(c3_b, np.float32), 2).reshape(64, 1)

    # phase-B packed consts (same for every core except extrasT)
    cb0 = np.zeros((128, CB_W), np.float32)
    w1eT = np.asarray(w1, np.float32)[:, 32768:].T  # [18, 304]
    w2T = np.asarray(w2, np.float32).T  # [304, 176]
    w3T = np.asarray(w3, np.float32).T  # [176, 48]
    cb0[0:18, 512:816] = w1eT
    cb0[0:128, 816:992] = w2T[0:128]
    cb0[0:128, 992:1168] = w2T[128:256]
    cb0[0:48, 1168:1344] = w2T[256:304]
    cb0[0:128, 1344:1392] = w3T[0:128]
    cb0[0:48, 1392:1440] = w3T[128:176]
    cb0[0:128, 1440:1441] = np.asarray(b2, np.float32)[0:128].reshape(128, 1)
    cb0[0:48, 1441:1442] = np.asarray(b2, np.float32)[128:176].reshape(48, 1)
    cb0[0:48, 1442:1443] = np.asarray(b3, np.float32).reshape(48, 1)

    w1bigT = np.ascontiguousarray(np.asarray(w1, np.float32)[:, :32768].T)  # [32768, 304]
    chunks = x.reshape(G, CS, G, CS).transpose(0, 2, 1, 3).reshape(G * G, CS * CS)
    pi = (np.arange(G * G) // G).astype(np.float32) * CS
    pj = (np.arange(G * G) % G).astype(np.float32) * CS

    P = np.arange(128)
    B = np.arange(32)
    c1b = np.asarray(c1_b, np.float32)
    c2b = np.asarray(c2_b, np.float32)

    maps_a, maps_b = [], []
    for r in range(NCORES):
        # xs: x rows [32r-7, 32r+43), cols padded by 1 each side
        xs = np.zeros((50, 258), np.float32)
        lo = 32 * r - 7
        hi = 32 * r + 43
        slo, shi = max(lo, 0), min(hi, IMG)
        xs[slo - lo : shi - lo, 1:257] = x[slo:shi, :]
        xs2 = np.ascontiguousarray(xs[16:50])  # [34, 258]

        ca = np.zeros((96, CA_W), np.float32)
        ca[0:18, 0:384] = s1
        ca[32:50, 0:384] = s1  # duplicate for the base-32 conv1 window
        # row-validity masks (zero out-of-image "phantom" pooled rows)
        for b in range(3):
            for jp in range(8):
                valid = 0 <= (16 * r - 3 + 8 * b + jp) < 128
                ca[jp * 8 : jp * 8 + 8, 384 + b] = 1.0 if valid else 0.0
                ca[jp * 8 : jp * 8 + 8, 387 + b] = c1b if valid else 0.0
            for jp in range(4):
                valid = 0 <= (8 * r - 1 + 4 * b + jp) < 64
                ca[jp * 16 : jp * 16 + 16, 390 + b] = 1.0 if valid else 0.0
                ca[jp * 16 : jp * 16 + 16, 393 + b] = c2b if valid else 0.0
        ca[0:64, 396:397] = bc3
        ca[0:80, 397:781] = s2
        ca[0:96, 781:1165] = s3

        # w1ps [128, 32*304]: w1ps[p, j*304+o] = w1[o, kglobal(p, j)],
        # kglobal = (p%32)*1024 + (4r + p//32)*32 + j
        kg = (P[None, :] % 32) * 1024 + (4 * r + P[None, :] // 32) * 32 + B[:, None]
        w1ps = np.ascontiguousarray(
            w1bigT[kg.ravel()].reshape(32, 128, 304).transpose(1, 0, 2).reshape(128, 32 * 304)
        )
        maps_a.append({"xs": xs, "xs2": xs2, "ca": ca, "w1ps": w1ps})

        cb = cb0.copy()
        sl = slice(512 * r, 512 * (r + 1))
        cb[0, 0:512] = pi[sl]
        cb[1, 0:512] = pj[sl]
        cb[2:18, 0:512] = chunks[sl].T
        maps_b.append({"cb": cb})
    return maps_a, maps_b, np.asarray(b1, np.float32)


def _mk_nc():
    import concourse.bacc as bacc

    # Bacc (not raw Bass): its compile() runs move_matmul_waits_to_ldweights /
    # generate_event_semaphores, required for the 1-wait-per-instruction
    # hardware constraint.
    return bacc.Bacc("TRN2", target_bir_lowering=False, debug=False, num_devices=NCORES)


def _build_phase_a(dt_mv, dt_cv):
    """Convs + sharded shared-matvec partial. Output: part [1, 304]."""
    import concourse.tile as tile
    from concourse import mybir

    f32 = mybir.dt.float32
    AF = mybir.ActivationFunctionType
    nc = _mk_nc()

    def din(name, shape):
        return nc.dram_tensor(name, list(shape), f32, kind="ExternalInput").ap()

    xs_d = din("xs", (50, 258))
    xs2_d = din("xs2", (34, 258))
    ca_d = din("ca", (96, CA_W))
    w1ps_d = din("w1ps", (128, 32 * 304))
    part_d = nc.dram_tensor("part", [1, 304], f32, kind="ExternalOutput").ap()

    with tile.TileContext(nc) as tc, ExitStack() as ctx:
        cpool = ctx.enter_context(tc.tile_pool(name="consts", bufs=1))
        spool = ctx.enter_context(tc.tile_pool(name="work", bufs=2))
        pconv = ctx.enter_context(tc.tile_pool(name="pconv", bufs=3, space="PSUM"))
        pmv = ctx.enter_context(tc.tile_pool(name="pmv", bufs=1, space="PSUM"))

        def _v(ap, dt):
            return ap if dt == f32 else ap.bitcast(dt)

        # warm the ScalarE activation-function table early (overlaps DMAs)
        scr = cpool.tile([1, 1], f32, tag="scr")
        nc.vector.memset(scr[:], 0.0)
        scr2 = cpool.tile([1, 1], f32, tag="scr2")
        nc.scalar.copy(scr2[:], scr[:])
        nc.scalar.activation(scr2[:], scr[:], AF.Relu)

        # small control inputs first so convs start immediately; conv1's
        # stationaries + masks (ca cols 0:397) land in their own small DMA
        ca_t = cpool.tile([96, CA_W], f32, tag="ca")
        nc.sync.dma_start(ca_t[:, 0:397], ca_d[:, 0:397])
        xs_t = cpool.tile([50, 258], f32, tag="xs")
        nc.sync.dma_start(xs_t[:], xs_d)
        xs2_t = cpool.tile([34, 258], f32, tag="xs2")
        nc.sync.dma_start(xs2_t[:], xs2_d)
        nc.sync.dma_start(ca_t[:, 397:CA_W], ca_d[:, 397:CA_W])

        # w1 stream: 4 chunk DMAs into one [128, 9728] tile, on the same SP
        # queue AFTER the control DMAs (queue FIFO keeps the small loads first)
        wst = cpool.tile([128, 32 * 304], f32, tag="w1s")
        CH = 4
        chw = 32 * 304 // CH
        for c in range(CH):
            nc.sync.dma_start(wst[:, c * chw : (c + 1) * chw], w1ps_d[:, c * chw : (c + 1) * chw])

        def s1ap(dx, base):  # stationary for conv1, at partition base 0 or 32
            return ca_t[base : base + 18, 128 * dx : 128 * (dx + 1)]

        def s2ap(dx):
            return ca_t[0:80, 397 + 128 * dx : 397 + 128 * (dx + 1)]

        def s3ap(dx):
            return ca_t[0:96, 781 + 128 * dx : 781 + 128 * (dx + 1)]

        mk1 = lambda b, n=64: ca_t[0:n, 384 + b : 385 + b]
        bm1 = lambda b, n=64: ca_t[0:n, 387 + b : 388 + b]
        mk2 = lambda b, n=64: ca_t[0:n, 390 + b : 391 + b]
        bm2 = lambda b, n=64: ca_t[0:n, 393 + b : 394 + b]
        bc3 = ca_t[0:64, 396:397]

        # next-layer moving-window tiles (built in place by ScalarE writes)
        m2 = [cpool.tile([80, 130], f32, tag=f"m2_{i}", name=f"m2_{i}") for i in range(3)]
        m3 = [cpool.tile([96, 66], f32, tag=f"m3_{i}", name=f"m3_{i}") for i in range(2)]
        xc_t = cpool.tile([128, 32], f32, tag="xc")
        for t in m2:
            nc.vector.memset(t[:], 0.0)
        for t in m3:
            nc.vector.memset(t[:], 0.0)

        def pool_to(ps, width):
            """psum [128, width] (m = (s, pair, c)) -> [64, width//2] max-pooled."""
            vtop = spool.tile([64, width], f32, tag=f"vt{width}")
            nc.scalar.copy(vtop[:], ps[0:64, :])
            v = spool.tile([64, width], f32, tag=f"v{width}")
            nc.vector.tensor_max(v[:], ps[64:128, :], vtop[:])
            vv = v[:].rearrange("p (x t) -> p x t", t=2)
            ph = spool.tile([64, width // 2], f32, tag=f"ph{width}")
            nc.vector.tensor_max(ph[:], vv[:, :, 0], vv[:, :, 1])
            return ph

        # ---- conv1: 3 blocks of 16 output rows -> M2 tiles
        win1 = [
            (xs_t[0:18, :], 0),
            (xs2_t[0:18, :], 0),
            (xs_t[32:50, :], 32),
        ]
        for b in range(3):
            rhs, base = win1[b]
            ps = pconv.tile([128, 256], f32, tag="cps")
            for dx in range(3):
                nc.tensor.matmul(
                    ps[:],
                    lhsT=_v(s1ap(dx, base), dt_cv),
                    rhs=_v(rhs[:, dx : dx + 256], dt_cv),
                    start=(dx == 0),
                    stop=(dx == 2),
                )
            ph = pool_to(ps, 256)  # [64, 128]: partition = jp*8+c, row = 8b+jp
            nc.scalar.activation(
                m2[b][0:64, 1:129], ph[:], AF.Relu, bias=bm1(b), scale=mk1(b)
            )
            if b >= 1:  # rows 8b, 8b+1 also tail rows 8..10 of previous window
                nc.scalar.activation(
                    m2[b - 1][64:80, 1:129],
                    ph[0:16, :],
                    AF.Relu,
                    bias=bm1(b, 16),
                    scale=mk1(b, 16),
                )

        # ---- conv2: 3 blocks of 8 output rows -> M3 tiles
        for b in range(3):
            ps = pconv.tile([128, 128], f32, tag="cps")
            for dx in range(3):
                nc.tensor.matmul(
                    ps[:],
                    lhsT=_v(s2ap(dx), dt_cv),
                    rhs=_v(m2[b][:, dx : dx + 128], dt_cv),
                    start=(dx == 0),
                    stop=(dx == 2),
                )
            ph = pool_to(ps, 128)  # [64, 64]: partition = jp'*16+co, row = 4b+jp'
            if b == 0:
                nc.scalar.activation(m3[0][0:64, 1:65], ph[:], AF.Relu, bias=bm2(0), scale=mk2(0))
            elif b == 1:
                nc.scalar.activation(m3[1][0:64, 1:65], ph[:], AF.Relu, bias=bm2(1), scale=mk2(1))
                nc.scalar.activation(
                    m3[0][64:96, 1:65], ph[0:32, :], AF.Relu, bias=bm2(1, 32), scale=mk2(1, 32)
                )
            else:
                nc.scalar.activation(
                    m3[1][64:96, 1:65], ph[0:32, :], AF.Relu, bias=bm2(2, 32), scale=mk2(2, 32)
                )

        # ---- conv3: 2 m-blocks of 4 output rows -> xc [128, 32]
        for g in range(2):
            ps = pconv.tile([128, 64], f32, tag="cps")
            for dx in range(3):
                nc.tensor.matmul(
                    ps[:],
                    lhsT=_v(s3ap(dx), dt_cv),
                    rhs=_v(m3[g][:, dx : dx + 64], dt_cv),
                    start=(dx == 0),
                    stop=(dx == 2),
                )
            ph = pool_to(ps, 64)  # [64, 32]
            nc.scalar.activation(xc_t[64 * g : 64 * g + 64, :], ph[:], AF.Relu, bias=bc3)

        # ---- shared matvec partial [1, 304]
        ps_mv = pmv.tile([1, 304], f32, tag="mv")
        for b in range(32):
            nc.tensor.matmul(
                ps_mv[:],
                lhsT=_v(xc_t[:, b : b + 1], dt_mv),
                rhs=_v(wst[:, 304 * b : 304 * (b + 1)], dt_mv),
                start=(b == 0),
                stop=(b == 31),
            )
        part_s = spool.tile([1, 304], f32, tag="part")
        nc.scalar.copy(part_s[:], ps_mv[:])
        nc.sync.dma_start(part_d, part_s[:])

    nc.compile()
    return nc


def _build_phase_b(dt_fc):
    """Patch FC for this core's 512 patches, given summed shared vector."""
    import concourse.tile as tile
    from concourse import mybir

    f32 = mybir.dt.float32
    AF = mybir.ActivationFunctionType
    nc = _mk_nc()

    cb_d = nc.dram_tensor("cb", [128, CB_W], f32, kind="ExternalInput").ap()
    shc_d = nc.dram_tensor("shc", [128, 3], f32, kind="ExternalInput").ap()
    yout_d = nc.dram_tensor("yout", [48, 512], f32, kind="ExternalOutput").ap()

    mblk = [(0, 128), (128, 128), (256, 48)]
    qblk = [(0, 128), (128, 48)]

    with tile.TileContext(nc) as tc, ExitStack() as ctx:
        cpool = ctx.enter_context(tc.tile_pool(name="consts", bufs=1))
        fpool = ctx.enter_context(tc.tile_pool(name="fc", bufs=1))
        pfc = ctx.enter_context(tc.tile_pool(name="pfc", bufs=1, space="PSUM"))
        phh = ctx.enter_context(tc.tile_pool(name="phh", bufs=3, space="PSUM"))

        def _v(ap, dt):
            return ap if dt == f32 else ap.bitcast(dt)

        # warm the ScalarE activation-function table early (overlaps DMAs)
        scr = cpool.tile([1, 1], f32, tag="scr")
        nc.vector.memset(scr[:], 0.0)
        scr2 = cpool.tile([1, 1], f32, tag="scr2")
        nc.scalar.activation(scr2[:], scr[:], AF.Relu)
        nc.scalar.activation(scr2[:], scr[:], AF.Sigmoid)

        cb = cpool.tile([128, CB_W], f32, tag="cb")
        nc.sync.dma_start(cb[:, 0:816], cb_d[:, 0:816])
        shc = cpool.tile([128, 3], f32, tag="shc")
        nc.scalar.dma_start(shc[:], shc_d)
        nc.scalar.dma_start(cb[:, 816:CB_W], cb_d[:, 816:CB_W])

        extrasT = cb[0:18, 0:512]
        w1eT = cb[0:18, 512:816]
        w2T_t = [cb[0:128, 816:992], cb[0:128, 992:1168], cb[0:48, 1168:1344]]
        w3T_t = [cb[0:128, 1344:1392], cb[0:48, 1392:1440]]
        b2c_t = [cb[0:128, 1440:1441], cb[0:48, 1441:1442]]
        b3c_t = cb[0:48, 1442:1443]
        sh_t = [shc[0:128, 0:1], shc[0:128, 1:2], shc[0:48, 2:3]]

        h1_t = []
        for i, (off, mb) in enumerate(mblk):
            ps_e = pfc.tile([mb, 512], f32, tag=f"pse{i}")
            nc.tensor.matmul(
                ps_e[:],
                lhsT=_v(w1eT[:, off : off + mb], dt_fc),
                rhs=_v(extrasT, dt_fc),
                start=True,
                stop=True,
            )
            h1 = fpool.tile([mb, 512], f32, tag=f"h1{i}")
            from concourse import mybir as _mb
            nc.vector.tensor_scalar(h1[:], ps_e[:], sh_t[i], 0.0, _mb.AluOpType.add, _mb.AluOpType.max)
            h1_t.append(h1)

        h2_t = []
        for q, (qoff, mq) in enumerate(qblk):
            ps_h = phh.tile([mq, 512], f32, tag="psh")
            for i, (off, mb) in enumerate(mblk):
                nc.tensor.matmul(
                    ps_h[:],
                    lhsT=_v(w2T_t[i][:, qoff : qoff + mq], dt_fc),
                    rhs=_v(h1_t[i][:], dt_fc),
                    start=(i == 0),
                    stop=(i == 2),
                )
            h2 = fpool.tile([mq, 512], f32, tag=f"h2{q}")
            nc.scalar.activation(h2[:], ps_h[:], AF.Relu, bias=b2c_t[q])
            h2_t.append(h2)

        ps_o = phh.tile([48, 512], f32, tag="psh")
        for q, (qoff, mq) in enumerate(qblk):
            nc.tensor.matmul(
                ps_o[:],
                lhsT=_v(w3T_t[q], dt_fc),
                rhs=_v(h2_t[q][:], dt_fc),
                start=(q == 0),
                stop=(q == 1),
            )
        outs = fpool.tile([48, 512], f32, tag="outs")
        nc.scalar.activation(outs[:], ps_o[:], AF.Sigmoid, bias=b3c_t)
        nc.sync.dma_start(yout_d, outs[:])

    nc.compile()
    return nc


def _shc_pack(sh):
    shc = np.zeros((128, 3), np.float32)
    shc[0:128, 0] = sh[0:128]
    shc[0:128, 1] = sh[128:256]
    shc[0:48, 2] = sh[256:304]
    return shc


def _run(maps_a, maps_b, b1, dt_mv, dt_fc, dt_cv, trace=False, trace_cores=None):
    from concourse.bass_utils import run_bass_kernel_spmd

    nca = _build_phase_a(dt_mv=dt_mv, dt_cv=dt_cv)
    res_a = run_bass_kernel_spmd(
        nca, maps_a, list(range(NCORES)), trace=trace, trace_cores=trace_cores
    )
    sh = np.sum([res_a.results[r]["part"][0] for r in range(NCORES)], axis=0) + b1
    shc = _shc_pack(sh)
    for mb in maps_b:
        mb["shc"] = shc
    ncb = _build_phase_b(dt_fc=dt_fc)
    res_b = run_bass_kernel_spmd(
        ncb, maps_b, list(range(NCORES)), trace=trace, trace_cores=trace_cores
    )
    full = np.empty((G * G, OUT), np.float32)
    for r in range(NCORES):
        full[512 * r : 512 * (r + 1), :] = res_b.results[r]["yout"].T
    return full.reshape(3, IMG, IMG), res_a, res_b


def kernel(**inputs):
    global DT_MV, DT_FC, DT_CV
    from concourse import mybir

    f32 = mybir.dt.float32
    if DT_MV is None:
        DT_MV = f32
    if DT_FC is None:
        DT_FC = f32
    if DT_CV is None:
        DT_CV = f32
    maps_a, maps_b, b1 = _host_inputs(**inputs)
    out, _, _ = _run(maps_a, maps_b, b1, DT_MV, DT_FC, DT_CV)
    return out


if __name__ == "__main__":
    import reference

    inp = {k: np.asarray(v) for k, v in reference.setup_inputs().items()}
    got = kernel(**inp)
    exp = np.asarray(reference.reference(**reference.setup_inputs()))
    err = np.abs(got - exp).max() / max(np.abs(exp).max(), 1e-9)
    print("Relative error:", err)



# revision 2
# speedup vs baseline: 1.6201x; 1.6201x over previous
"""ColorizationNet Trainium2 kernel (8 NeuronCores, SPMD, two phases).

Structure exploited: rows of the big FC input [4096, 32786] share an identical
x_conv prefix (32768 cols), so

    fc_in @ w1.T = x_conv @ w1[:, :32768].T  (one shared matvec, [304])
                 + [pos|chunks] @ w1[:, 32768:].T  ([4096,18] GEMM)

Sharding (core r of 8):
  - conv backbone row-sharded: core r produces the x_conv slice for pooled
    rows [4r, 4r+4) of every channel (halos via zero-padded input windows,
    out-of-image "phantom" rows masked to zero via activation scale).
  - shared matvec K-sharded to match (each core streams 1/8 of w1's big
    part, 2.5 MB bf16, laid out so each SBUF partition's data is contiguous
    in DRAM).  Phase A outputs the 8 partials [304]; the host sums them
    (collectives are unavailable under the axon PJRT execution path).
  - phase B: patch FC sharded by patch row, core r handles patches
    [512r, 512(r+1)).

All matmul operands are bf16 (1 cycle/row on PE at any free size, vs 4 for
fp32); accumulation is fp32 in PSUM, and activation bias/scale constants
stay fp32.  Convs use a banded-rows formulation: moving operand = input
rows on SBUF partitions, stationary = banded weight matrix with output
columns m = (s, rowpair, c_out) so the 2x2 maxpool's vertical pair is
partitions p / p+64 (one tensor_max) and the horizontal pair is a stride-2
free-dim pair.  Each layer's pooled activation is written by ScalarE
directly into the next layer's moving-window tiles (no DRAM round trips).
"""

import sys

for _p in ("/opt/trn_rl_repo",):
    if _p not in sys.path:
        sys.path.insert(0, _p)

import numpy as np
import ml_dtypes
from contextlib import ExitStack

BF16 = ml_dtypes.bfloat16

IMG = 256
CS = 4
G = 64
H1 = 304
H2 = 176
OUT = 48
NCORES = 8

# phase-A packed bf16 weights: [96, 1152]
#   s1 [18, 384] @0 (+ copy at rows 32..50), s2 [80, 384] @384, s3 [96, 384] @768
# phase-A packed fp32 masks/biases: [96, 13]
#   mk1 [64,3] @0, bm1 [64,3] @3, mk2 [64,3] @6, bm2 [64,3] @9, bc3 [64,1] @12
CAW_W = 1152
CAM_W = 13

# phase-B packed bf16 consts: [128, 1440]
#   extrasT [18, 512] @0, w1eT [18, 304] @512, w2a/b [128, 176] @816/@992,
#   w2c [48, 176] @1168, w3a [128, 48] @1344, w3b [48, 48] @1392
# phase-B fp32 biases: [128, 3]: b2a col 0, b2b col 1, b3 col 2
CBW_W = 1440
CBB_W = 3


def _build_s1(c1_w):
    # [18, 3, 128]: rows i = in-row in window; cols m = s*64 + jp*8 + c
    s1 = np.zeros((18, 3, 128), np.float32)
    for dx in range(3):
        for s in range(2):
            for jp in range(8):
                j = 2 * jp + s
                for c in range(8):
                    m = s * 64 + jp * 8 + c
                    for dy in range(3):
                        s1[j + dy, dx, m] = c1_w[c, 0, dy, dx]
    return np.ascontiguousarray(s1.reshape(18, 3 * 128))


def _build_s2(c2_w):
    # [80, 3, 128]: rows k = delta*8 + ci (ci in 0..8); cols m = s*64+jp*16+co
    s2 = np.zeros((80, 3, 128), np.float32)
    for dx in range(3):
        for s in range(2):
            for jp in range(4):
                j2 = 2 * jp + s
                for co in range(16):
                    m = s * 64 + jp * 16 + co
                    for ci in range(8):
                        for dy in range(3):
                            s2[(j2 + dy) * 8 + ci, dx, m] = c2_w[co, ci, dy, dx]
    return np.ascontiguousarray(s2.reshape(80, 3 * 128))


def _build_s3(c3_w):
    # [96, 3, 128]: rows k = delta*16 + ci (ci in 0..16); cols m = s*64+jpp*32+co
    s3 = np.zeros((96, 3, 128), np.float32)
    for dx in range(3):
        for s in range(2):
            for jpp in range(2):
                j3 = 2 * jpp + s
                for co in range(32):
                    m = s * 64 + jpp * 32 + co
                    for ci in range(16):
                        for dy in range(3):
                            s3[(j3 + dy) * 16 + ci, dx, m] = c3_w[co, ci, dy, dx]
    return np.ascontiguousarray(s3.reshape(96, 3 * 128))


def _host_inputs(x, c1_w, c1_b, c2_w, c2_b, c3_w, c3_b, w1, b1, w2, b2, w3, b3):
    """Returns (in_maps_a, in_maps_b_partial, b1). Each phase-A map has
    'xs' [50,258] bf16, 'xs2' [34,258] bf16, 'caw' [96, CAW_W] bf16,
    'cam' [96, CAM_W] f32, 'w1ps' [128, 9728] bf16.  Each phase-B map has
    'cbw' [128, CBW_W] bf16 and 'cbb' [128, CBB_W] f32; 'shc' [128, 3] f32
    is added after phase A."""
    x = np.asarray(x, np.float32).reshape(IMG, IMG)
    s1 = _build_s1(np.asarray(c1_w, np.float32))
    s2 = _build_s2(np.asarray(c2_w, np.float32))
    s3 = _build_s3(np.asarray(c3_w, np.float32))
    bc3 = np.tile(np.asarray(c3_b, np.float32), 2).reshape(64, 1)

    caw0 = np.zeros((96, CAW_W), np.float32)
    caw0[0:18, 0:384] = s1
    caw0[32:50, 0:384] = s1  # duplicate for the base-32 conv1 window
    caw0[0:80, 384:768] = s2
    caw0[0:96, 768:1152] = s3
    caw0 = caw0.astype(BF16)

    # phase-B packed consts (same for every core except extrasT)
    cbw0 = np.zeros((128, CBW_W), np.float32)
    w1eT = np.asarray(w1, np.float32)[:, 32768:].T  # [18, 304]
    w2T = np.asarray(w2, np.float32).T  # [304, 176]
    w3T = np.asarray(w3, np.float32).T  # [176, 48]
    cbw0[0:18, 512:816] = w1eT
    cbw0[0:128, 816:992] = w2T[0:128]
    cbw0[0:128, 992:1168] = w2T[128:256]
    cbw0[0:48, 1168:1344] = w2T[256:304]
    cbw0[0:128, 1344:1392] = w3T[0:128]
    cbw0[0:48, 1392:1440] = w3T[128:176]
    cbb0 = np.zeros((128, CBB_W), np.float32)
    cbb0[0:128, 0] = np.asarray(b2, np.float32)[0:128]
    cbb0[0:48, 1] = np.asarray(b2, np.float32)[128:176]
    cbb0[0:48, 2] = np.asarray(b3, np.float32)

    w1bigT = np.ascontiguousarray(np.asarray(w1, np.float32)[:, :32768].T)  # [32768, 304]
    chunks = x.reshape(G, CS, G, CS).transpose(0, 2, 1, 3).reshape(G * G, CS * CS)
    pi = (np.arange(G * G) // G).astype(np.float32) * CS
    pj = (np.arange(G * G) % G).astype(np.float32) * CS

    P = np.arange(128)
    B = np.arange(32)
    c1b = np.asarray(c1_b, np.float32)
    c2b = np.asarray(c2_b, np.float32)

    maps_a, maps_b = [], []
    for r in range(NCORES):
        # xs: x rows [32r-7, 32r+43), cols padded by 1 each side
        xs = np.zeros((50, 258), np.float32)
        lo = 32 * r - 7
        hi = 32 * r + 43
        slo, shi = max(lo, 0), min(hi, IMG)
        xs[slo - lo : shi - lo, 1:257] = x[slo:shi, :]
        xs = xs.astype(BF16)
        xs2 = np.ascontiguousarray(xs[16:50])  # [34, 258]

        cam = np.zeros((96, CAM_W), np.float32)
        # row-validity masks (zero out-of-image "phantom" pooled rows)
        for b in range(3):
            for jp in range(8):
                valid = 0 <= (16 * r - 3 + 8 * b + jp) < 128
                cam[jp * 8 : jp * 8 + 8, 0 + b] = 1.0 if valid else 0.0
                cam[jp * 8 : jp * 8 + 8, 3 + b] = c1b if valid else 0.0
            for jp in range(4):
                valid = 0 <= (8 * r - 1 + 4 * b + jp) < 64
                cam[jp * 16 : jp * 16 + 16, 6 + b] = 1.0 if valid else 0.0
                cam[jp * 16 : jp * 16 + 16, 9 + b] = c2b if valid else 0.0
        cam[0:64, 12:13] = bc3

        # w1ps [128, 32*304]: w1ps[p, j*304+o] = w1[o, kglobal(p, j)],
        # kglobal = (p%32)*1024 + (4r + p//32)*32 + j
        kg = (P[None, :] % 32) * 1024 + (4 * r + P[None, :] // 32) * 32 + B[:, None]
        w1ps = np.ascontiguousarray(
            w1bigT[kg.ravel()].reshape(32, 128, 304).transpose(1, 0, 2).reshape(128, 32 * 304)
        ).astype(BF16)
        maps_a.append({"xs": xs, "xs2": xs2, "caw": caw0, "cam": cam, "w1ps": w1ps})

        cbw = cbw0.copy()
        sl = slice(512 * r, 512 * (r + 1))
        cbw[0, 0:512] = pi[sl]
        cbw[1, 0:512] = pj[sl]
        cbw[2:18, 0:512] = chunks[sl].T
        maps_b.append({"cbw": cbw.astype(BF16), "cbb": cbb0})
    return maps_a, maps_b, np.asarray(b1, np.float32)


def _mk_nc():
    import concourse.bacc as bacc

    # Bacc (not raw Bass): its compile() runs move_matmul_waits_to_ldweights /
    # generate_event_semaphores, required for the 1-wait-per-instruction
    # hardware constraint.
    return bacc.Bacc("TRN2", target_bir_lowering=False, debug=False, num_devices=NCORES)


def _build_phase_a():
    """Convs + sharded shared-matvec partial. Output: part [1, 304]."""
    import concourse.tile as tile
    from concourse import mybir

    f32 = mybir.dt.float32
    bf16 = mybir.dt.bfloat16
    AF = mybir.ActivationFunctionType
    nc = _mk_nc()

    def din(name, shape, dt=f32):
        return nc.dram_tensor(name, list(shape), dt, kind="ExternalInput").ap()

    xs_d = din("xs", (50, 258), bf16)
    xs2_d = din("xs2", (34, 258), bf16)
    caw_d = din("caw", (96, CAW_W), bf16)
    cam_d = din("cam", (96, CAM_W), f32)
    w1ps_d = din("w1ps", (128, 32 * 304), bf16)
    part_d = nc.dram_tensor("part", [1, 304], f32, kind="ExternalOutput").ap()

    with tile.TileContext(nc) as tc, ExitStack() as ctx:
        cpool = ctx.enter_context(tc.tile_pool(name="consts", bufs=1))
        spool = ctx.enter_context(tc.tile_pool(name="work", bufs=2))
        pconv = ctx.enter_context(tc.tile_pool(name="pconv", bufs=3, space="PSUM"))
        pmv = ctx.enter_context(tc.tile_pool(name="pmv", bufs=1, space="PSUM"))

        # warm the ScalarE activation-function table early (overlaps DMAs)
        scr = cpool.tile([1, 1], f32, tag="scr")
        nc.vector.memset(scr[:], 0.0)
        scr2 = cpool.tile([1, 1], f32, tag="scr2")
        nc.scalar.copy(scr2[:], scr[:])
        nc.scalar.activation(scr2[:], scr[:], AF.Relu)

        # small control inputs first so convs start immediately
        cam_t = cpool.tile([96, CAM_W], f32, tag="cam")
        nc.sync.dma_start(cam_t[:], cam_d)
        caw_t = cpool.tile([96, CAW_W], bf16, tag="caw")
        nc.sync.dma_start(caw_t[:, 0:384], caw_d[:, 0:384])
        xs_t = cpool.tile([50, 258], bf16, tag="xs")
        nc.sync.dma_start(xs_t[:], xs_d)
        xs2_t = cpool.tile([34, 258], bf16, tag="xs2")
        nc.sync.dma_start(xs2_t[:], xs2_d)
        nc.sync.dma_start(caw_t[:, 384:CAW_W], caw_d[:, 384:CAW_W])

        # w1 stream: 4 chunk DMAs into one [128, 9728] tile, on the same SP
        # queue AFTER the control DMAs (queue FIFO keeps the small loads first)
        wst = cpool.tile([128, 32 * 304], bf16, tag="w1s")
        CH = 4
        chw = 32 * 304 // CH
        for c in range(CH):
            nc.sync.dma_start(wst[:, c * chw : (c + 1) * chw], w1ps_d[:, c * chw : (c + 1) * chw])

        def s1ap(dx, base):  # stationary for conv1, at partition base 0 or 32
            return caw_t[base : base + 18, 128 * dx : 128 * (dx + 1)]

        def s2ap(dx):
            return caw_t[0:80, 384 + 128 * dx : 384 + 128 * (dx + 1)]

        def s3ap(dx):
            return caw_t[0:96, 768 + 128 * dx : 768 + 128 * (dx + 1)]

        mk1 = lambda b, n=64: cam_t[0:n, 0 + b : 1 + b]
        bm1 = lambda b, n=64: cam_t[0:n, 3 + b : 4 + b]
        mk2 = lambda b, n=64: cam_t[0:n, 6 + b : 7 + b]
        bm2 = lambda b, n=64: cam_t[0:n, 9 + b : 10 + b]
        bc3 = cam_t[0:64, 12:13]

        # next-layer moving-window tiles (built in place by ScalarE writes)
        m2 = [cpool.tile([80, 130], bf16, tag=f"m2_{i}", name=f"m2_{i}") for i in range(3)]
        m3 = [cpool.tile([96, 66], bf16, tag=f"m3_{i}", name=f"m3_{i}") for i in range(2)]
        xc_t = cpool.tile([128, 32], bf16, tag="xc")
        for t in m2:
            nc.vector.memset(t[:], 0.0)
        for t in m3:
            nc.vector.memset(t[:], 0.0)

        def pool_to(ps, width):
            """psum [128, width] (m = (s, pair, c)) -> [64, width//2] max-pooled."""
            vtop = spool.tile([64, width], f32, tag=f"vt{width}")
            nc.scalar.copy(vtop[:], ps[0:64, :])
            v = spool.tile([64, width], f32, tag=f"v{width}")
            nc.vector.tensor_max(v[:], ps[64:128, :], vtop[:])
            vv = v[:].rearrange("p (x t) -> p x t", t=2)
            ph = spool.tile([64, width // 2], f32, tag=f"ph{width}")
            nc.vector.tensor_max(ph[:], vv[:, :, 0], vv[:, :, 1])
            return ph

        # ---- conv1: 3 blocks of 16 output rows -> M2 tiles
        win1 = [
            (xs_t[0:18, :], 0),
            (xs2_t[0:18, :], 0),
            (xs_t[32:50, :], 32),
        ]
        for b in range(3):
            rhs, base = win1[b]
            ps = pconv.tile([128, 256], f32, tag="cps")
            for dx in range(3):
                nc.tensor.matmul(
                    ps[:],
                    lhsT=s1ap(dx, base),
                    rhs=rhs[:, dx : dx + 256],
                    start=(dx == 0),
                    stop=(dx == 2),
                )
            ph = pool_to(ps, 256)  # [64, 128]: partition = jp*8+c, row = 8b+jp
            nc.scalar.activation(
                m2[b][0:64, 1:129], ph[:], AF.Relu, bias=bm1(b), scale=mk1(b)
            )
            if b >= 1:  # rows 8b, 8b+1 also tail rows 8..10 of previous window
                nc.scalar.activation(
                    m2[b - 1][64:80, 1:129],
                    ph[0:16, :],
                    AF.Relu,
                    bias=bm1(b, 16),
                    scale=mk1(b, 16),
                )

        # ---- conv2: 3 blocks of 8 output rows -> M3 tiles
        for b in range(3):
            ps = pconv.tile([128, 128], f32, tag="cps")
            for dx in range(3):
                nc.tensor.matmul(
                    ps[:],
                    lhsT=s2ap(dx),
                    rhs=m2[b][:, dx : dx + 128],
                    start=(dx == 0),
                    stop=(dx == 2),
                )
            ph = pool_to(ps, 128)  # [64, 64]: partition = jp'*16+co, row = 4b+jp'
            if b == 0:
                nc.scalar.activation(m3[0][0:64, 1:65], ph[:], AF.Relu, bias=bm2(0), scale=mk2(0))
            elif b == 1:
                nc.scalar.activation(m3[1][0:64, 1:65], ph[:], AF.Relu, bias=bm2(1), scale=mk2(1))
                nc.scalar.activation(
                    m3[0][64:96, 1:65], ph[0:32, :], AF.Relu, bias=bm2(1, 32), scale=mk2(1, 32)
                )
            else:
                nc.scalar.activation(
                    m3[1][64:96, 1:65], ph[0:32, :], AF.Relu, bias=bm2(2, 32), scale=mk2(2, 32)
                )

        # ---- conv3: 2 m-blocks of 4 output rows -> xc [128, 32]
        for g in range(2):
            ps = pconv.tile([128, 64], f32, tag="cps")
            for dx in range(3):
                nc.tensor.matmul(
                    ps[:],
                    lhsT=s3ap(dx),
                    rhs=m3[g][:, dx : dx + 64],
                    start=(dx == 0),
                    stop=(dx == 2),
                )
            ph = pool_to(ps, 64)  # [64, 32]
            nc.scalar.activation(xc_t[64 * g : 64 * g + 64, :], ph[:], AF.Relu, bias=bc3)

        # ---- shared matvec partial [1, 304]
        ps_mv = pmv.tile([1, 304], f32, tag="mv")
        for b in range(32):
            nc.tensor.matmul(
                ps_mv[:],
                lhsT=xc_t[:, b : b + 1],
                rhs=wst[:, 304 * b : 304 * (b + 1)],
                start=(b == 0),
                stop=(b == 31),
            )
        part_s = spool.tile([1, 304], f32, tag="part")
        nc.scalar.copy(part_s[:], ps_mv[:])
        nc.sync.dma_start(part_d, part_s[:])

    nc.compile()
    return nc


def _build_phase_b():
    """Patch FC for this core's 512 patches, given summed shared vector."""
    import concourse.tile as tile
    from concourse import mybir

    f32 = mybir.dt.float32
    bf16 = mybir.dt.bfloat16
    AF = mybir.ActivationFunctionType
    nc = _mk_nc()

    cbw_d = nc.dram_tensor("cbw", [128, CBW_W], bf16, kind="ExternalInput").ap()
    cbb_d = nc.dram_tensor("cbb", [128, CBB_W], f32, kind="ExternalInput").ap()
    shc_d = nc.dram_tensor("shc", [128, 3], f32, kind="ExternalInput").ap()
    yout_d = nc.dram_tensor("yout", [48, 512], f32, kind="ExternalOutput").ap()

    mblk = [(0, 128), (128, 128), (256, 48)]
    qblk = [(0, 128), (128, 48)]

    with tile.TileContext(nc) as tc, ExitStack() as ctx:
        cpool = ctx.enter_context(tc.tile_pool(name="consts", bufs=1))
        fpool = ctx.enter_context(tc.tile_pool(name="fc", bufs=1))
        pfc = ctx.enter_context(tc.tile_pool(name="pfc", bufs=1, space="PSUM"))
        phh = ctx.enter_context(tc.tile_pool(name="phh", bufs=3, space="PSUM"))

        # warm the ScalarE activation-function table early (overlaps DMAs)
        scr = cpool.tile([1, 1], f32, tag="scr")
        nc.vector.memset(scr[:], 0.0)
        scr2 = cpool.tile([1, 1], f32, tag="scr2")
        nc.scalar.activation(scr2[:], scr[:], AF.Relu)
        nc.scalar.activation(scr2[:], scr[:], AF.Sigmoid)

        cbw = cpool.tile([128, CBW_W], bf16, tag="cbw")
        nc.sync.dma_start(cbw[:, 0:816], cbw_d[:, 0:816])
        cbb = cpool.tile([128, CBB_W], f32, tag="cbb")
        nc.scalar.dma_start(cbb[:], cbb_d)
        shc = cpool.tile([128, 3], f32, tag="shc")
        nc.scalar.dma_start(shc[:], shc_d)
        nc.scalar.dma_start(cbw[:, 816:CBW_W], cbw_d[:, 816:CBW_W])

        extrasT = cbw[0:18, 0:512]
        w1eT = cbw[0:18, 512:816]
        w2T_t = [cbw[0:128, 816:992], cbw[0:128, 992:1168], cbw[0:48, 1168:1344]]
        w3T_t = [cbw[0:128, 1344:1392], cbw[0:48, 1392:1440]]
        b2c_t = [cbb[0:128, 0:1], cbb[0:48, 1:2]]
        b3c_t = cbb[0:48, 2:3]
        sh_t = [shc[0:128, 0:1], shc[0:128, 1:2], shc[0:48, 2:3]]

        h1_t = []
        for i, (off, mb) in enumerate(mblk):
            ps_e = pfc.tile([mb, 512], f32, tag=f"pse{i}")
            nc.tensor.matmul(
                ps_e[:],
                lhsT=w1eT[:, off : off + mb],
                rhs=extrasT,
                start=True,
                stop=True,
            )
            h1 = fpool.tile([mb, 512], bf16, tag=f"h1{i}")
            from concourse import mybir as _mb
            nc.vector.tensor_scalar(h1[:], ps_e[:], sh_t[i], 0.0, _mb.AluOpType.add, _mb.AluOpType.max)
            h1_t.append(h1)

        h2_t = []
        for q, (qoff, mq) in enumerate(qblk):
            ps_h = phh.tile([mq, 512], f32, tag="psh")
            for i, (off, mb) in enumerate(mblk):
                nc.tensor.matmul(
                    ps_h[:],
                    lhsT=w2T_t[i][:, qoff : qoff + mq],
                    rhs=h1_t[i][:],
                    start=(i == 0),
                    stop=(i == 2),
                )
            h2 = fpool.tile([mq, 512], bf16, tag=f"h2{q}")
            nc.scalar.activation(h2[:], ps_h[:], AF.Relu, bias=b2c_t[q])
            h2_t.append(h2)

        ps_o = phh.tile([48, 512], f32, tag="psh")
        for q, (qoff, mq) in enumerate(qblk):
            nc.tensor.matmul(
                ps_o[:],
                lhsT=w3T_t[q],
                rhs=h2_t[q][:],
                start=(q == 0),
                stop=(q == 1),
            )
        outs = fpool.tile([48, 512], f32, tag="outs")
        nc.scalar.activation(outs[:], ps_o[:], AF.Sigmoid, bias=b3c_t)
        nc.sync.dma_start(yout_d, outs[:])

    nc.compile()
    return nc


def _shc_pack(sh):
    shc = np.zeros((128, 3), np.float32)
    shc[0:128, 0] = sh[0:128]
    shc[0:128, 1] = sh[128:256]
    shc[0:48, 2] = sh[256:304]
    return shc


def _run(maps_a, maps_b, b1, trace=False, trace_cores=None):
    from concourse.bass_utils import run_bass_kernel_spmd

    nca = _build_phase_a()
    res_a = run_bass_kernel_spmd(
        nca, maps_a, list(range(NCORES)), trace=trace, trace_cores=trace_cores
    )
    sh = np.sum([res_a.results[r]["part"][0] for r in range(NCORES)], axis=0) + b1
    shc = _shc_pack(sh)
    for mb in maps_b:
        mb["shc"] = shc
    ncb = _build_phase_b()
    res_b = run_bass_kernel_spmd(
        ncb, maps_b, list(range(NCORES)), trace=trace, trace_cores=trace_cores
    )
    full = np.empty((G * G, OUT), np.float32)
    for r in range(NCORES):
        full[512 * r : 512 * (r + 1), :] = res_b.results[r]["yout"].T
    return full.reshape(3, IMG, IMG), res_a, res_b


def kernel(**inputs):
    maps_a, maps_b, b1 = _host_inputs(**inputs)
    out, _, _ = _run(maps_a, maps_b, b1)
    return out


if __name__ == "__main__":
    import reference

    inp = {k: np.asarray(v) for k, v in reference.setup_inputs().items()}
    got = kernel(**inp)
    exp = np.asarray(reference.reference(**reference.setup_inputs()))
    err = np.abs(got - exp).max() / max(np.abs(exp).max(), 1e-9)
    print("Relative error:", err)
